# revision 10
# baseline (speedup 1.0000x reference)
"""LoRA linear kernel for Trainium2 (8 NeuronCores, SPMD data-parallel).

Computes y = x @ (B @ A)^T for
    x: [4, 2048, 4096] f32, B: [4096, 16] f32, A: [16, 4096] f32.

Strategy: never materialize W = B @ A.  Factor as t = x @ A^T (rank 16)
then y = t @ B^T.  Tokens (4*2048 = 8192) are sharded across 8 cores
(1024 tokens each); A and B are replicated.

The kernel is HBM-bandwidth bound (~358 GB/s/core), so x is staged and
y is returned in float16 (tolerance is 2e-2; fp16 end-to-end gives
~7e-4), halving HBM traffic vs fp32.  Matmuls run fp16 x fp16 with
fp32 PSUM accumulation; y is cast to fp16 during PSUM evacuation.

DMA-descriptor discipline (the descriptor structure follows the SBUF
tile's innermost contiguous run, so):
  - x is staged chunk-major and loaded into FLAT [128, 8*512] tiles
    -> 8 KiB per-partition descriptors (near line rate).
  - at is staged flat [128, 512] (not [128, 32, 16], which would emit
    4096 32-byte descriptors).
  - y rows are [128, 4096] -> 8 KiB descriptors.

PSUM evacuation: two mm2 n-slices per 2-bank PSUM tile, evacuated by
single [128, 1024] fp32->fp16 copies alternating DVE/ACT.  y DMAs are
issued from the Sync sequencer: an ACT-issued DMA would sit in ACT's
strict-FIFO queue waiting on DVE's copies, stalling ACT's own copies.

Per-core dataflow:
  mm1: t^T[16, tok]  = sum_ko  A^T[ko]  (lhsT [128,16]) . x^T[ko] (rhs [128,512])
  mm2: y[tok128, o]  = t^T[:, chunk] (lhsT [16,128])    . B^T     (rhs [16,512])
  y DMA'd out in natural token-major layout -> host just concatenates.
"""

import sys

import numpy as np

if "/opt/trn_rl_repo" not in sys.path:
    sys.path.insert(0, "/opt/trn_rl_repo")

# Problem shape (hardcoded per contract)
BATCH = 4
SEQ = 2048
D = 4096          # in_features == out_features
R = 16            # lora rank
NCORES = 8
NTOK = BATCH * SEQ            # 8192 tokens total
TOK = NTOK // NCORES          # 1024 tokens per core
P = 128                       # partitions
KO = D // P                   # 32 feature chunks
TB = 512                      # token block for mm1 (psum bank: 512 fp32)
NB = 512                      # mm2 free dim (psum bank: 512 fp32)
KC = 8                        # ko chunks per x DMA (8 KiB/partition)
NCH = KO // KC                # x DMA chunks per token block

# Module-level knobs for test.py (harness never touches these)
TRACE = False
LAST_RESULTS = None

_nc_cache = None


def _build_program():
    from concourse import bacc, mybir, tile

    # Bacc (not raw Bass): its finalize() runs generate_event_semaphores,
    # which splits multi-sem waits to satisfy TRN2's 1-wait-per-instruction
    # hardware constraint (walrus rejects >1 otherwise).
    nc = bacc.Bacc(
        "TRN2", target_bir_lowering=False, debug=False, num_devices=NCORES
    )

    f32 = mybir.dt.float32
    f16 = mybir.dt.float16

    n_blocks = TOK // TB
    xt = nc.dram_tensor(
        "xt", [n_blocks * NCH, P, KC * TB], f16, kind="ExternalInput"
    )
    at = nc.dram_tensor("at", [P, KO * R], f16, kind="ExternalInput")
    bt = nc.dram_tensor("bt", [R, D], f16, kind="ExternalInput")
    y = nc.dram_tensor("y", [TOK, D], f16, kind="ExternalOutput")

    with tile.TileContext(nc) as tc:
        with (
            tc.tile_pool(name="consts", bufs=1) as consts,
            tc.tile_pool(name="xin", bufs=2 * NCH) as xin,
            tc.tile_pool(name="tbuf", bufs=2) as tbuf,
            tc.tile_pool(name="yout", bufs=6) as yout,
            tc.tile_pool(name="pt", bufs=2, space="PSUM") as pt_pool,
            tc.tile_pool(name="py", bufs=3, space="PSUM") as py_pool,
        ):
            at_s = consts.tile([P, KO * R], f16)
            nc.sync.dma_start(at_s[:], at[:])
            bt_s = consts.tile([R, D], f16)
            nc.sync.dma_start(bt_s[:], bt[:])

            # Warm-up matmuls: (a) make PE observe the at/bt DMA sems early,
            # (b) keep PE streaming during the x-DMA prologue so the HAM
            # clock gate reaches K=8/8 before the real matmuls start.
            obs1 = py_pool.tile([R, R], f32, tag="psum_y")
            nc.tensor.matmul(obs1[:], at_s[:, :R], at_s[:, :R], start=True, stop=True)
            for _ in range(4):
                warm = py_pool.tile([P, 2, NB], f32, tag="psum_y")
                nc.tensor.matmul(warm[:, 0, :], bt_s[:, :P], bt_s[:, :NB], start=True, stop=True)
            tc.no_sync_barrier()

            def load_x(tb):
                xts = []
                for kc in range(NCH):
                    xt_tile = xin.tile([P, KC * TB], f16, tag="xt")
                    nc.sync.dma_start(xt_tile[:], xt[tb * NCH + kc])
                    xts.append(xt_tile)
                return xts

            def mm1(xts, psum_t):
                for kc in range(NCH):
                    for j in range(KC):
                        ko = kc * KC + j
                        nc.tensor.matmul(
                            psum_t[:],
                            at_s[:, ko * R : (ko + 1) * R],
                            xts[kc][:, j * TB : (j + 1) * TB],
                            start=(ko == 0),
                            stop=(ko == KO - 1),
                        )

            def round_t(psum_t):
                # fp32 PSUM -> fp16 SBUF: the mm2 stationary operand
                tT = tbuf.tile([R, TB], f16)
                nc.vector.tensor_copy(tT[:], psum_t[:])
                return tT

            def mm2_chunk(tb, c, tT):
                y_row = yout.tile([P, D], f16)
                for pair in range(D // (2 * NB)):
                    # Two n-slices into one 2-bank PSUM tile, evacuated by a
                    # single [128, 1024] fp32->fp16 copy (amortizes the
                    # fixed per-op PSUM-read cost).
                    psum_y = py_pool.tile([P, 2, NB], f32, tag="psum_y")
                    for k in range(2):
                        n = 2 * pair + k
                        nc.tensor.matmul(
                            psum_y[:, k, :],
                            tT[:, c * P : (c + 1) * P],
                            bt_s[:, n * NB : (n + 1) * NB],
                            start=True,
                            stop=True,
                        )
                    # Alternate PSUM-evacuation between DVE and ACT so
                    # neither engine gates the tensor engine's psum slots
                    n0 = 2 * pair * NB
                    if pair % 2 == 1:
                        nc.scalar.copy(y_row[:, n0 : n0 + 2 * NB], psum_y[:])
                    else:
                        nc.vector.tensor_copy(y_row[:, n0 : n0 + 2 * NB], psum_y[:])
                row0 = tb * TB + c * P
                # GpSimd (SWDGE) ring: x owns the Sync ring (a y DMA behind
                # 8 MB of queued x descriptors would serialize), and an
                # ACT-issued DMA would block ACT's own copies in its FIFO.
                nc.gpsimd.dma_start(y[row0 : row0 + P, :], y_row[:])

            def mm1_range(xts, psum_t, kc):
                for j in range(KC):
                    ko = kc * KC + j
                    nc.tensor.matmul(
                        psum_t[:],
                        at_s[:, ko * R : (ko + 1) * R],
                        xts[kc][:, j * TB : (j + 1) * TB],
                        start=(ko == 0),
                        stop=(ko == KO - 1),
                    )

            # PE order must follow x-arrival order (PE is FIFO: a matmul
            # waiting on a late DMA blocks everything behind it).  mm1 of
            # block b+1 is interleaved between mm2 chunks of block b so the
            # PE instruction stream stays dense (HAM stays warm) and block
            # b+1's t is ready the moment block b's y chunks finish.
            xts0 = load_x(0)
            xts1 = load_x(1)
            psum_t0 = pt_pool.tile([R, TB], f32, tag="psum_t")
            mm1(xts0, psum_t0)
            tT0 = round_t(psum_t0)
            psum_t1 = pt_pool.tile([R, TB], f32, tag="psum_t")
            for c in range(TB // P):
                mm2_chunk(0, c, tT0)
                mm1_range(xts1, psum_t1, c)
            tT1 = round_t(psum_t1)
            for c in range(TB // P):
                mm2_chunk(1, c, tT1)

    nc.finalize()
    return nc


def kernel(x, lora_matrix_B, lora_matrix_A):
    global _nc_cache, LAST_RESULTS
    from concourse.bass_utils import run_bass_kernel_spmd

    if _nc_cache is None:
        _nc_cache = _build_program()
    nc = _nc_cache

    x_flat = np.asarray(x, dtype=np.float32).reshape(NTOK, D).astype(np.float16)
    A = np.asarray(lora_matrix_A, dtype=np.float32).astype(np.float16)
    B = np.asarray(lora_matrix_B, dtype=np.float32).astype(np.float16)

    # at[p, ko*R + j] = A[j, ko*128 + p];  bt[j, o] = B[o, j]
    at_prep = np.ascontiguousarray(
        A.reshape(R, KO, P).transpose(2, 1, 0).reshape(P, KO * R)
    )
    bt_prep = np.ascontiguousarray(B.T)

    n_blocks = TOK // TB
    in_maps = []
    for c in range(NCORES):
        xc = x_flat[c * TOK : (c + 1) * TOK, :]
        # xt[(tb, kc), p, j*TB + t] = xc[tb*TB + t, (kc*KC + j)*128 + p]
        xt_prep = np.ascontiguousarray(
            xc.reshape(n_blocks, TB, NCH, KC, P).transpose(0, 2, 4, 3, 1)
        ).reshape(n_blocks * NCH, P, KC * TB)
        in_maps.append({"xt": xt_prep, "at": at_prep, "bt": bt_prep})

    res = run_bass_kernel_spmd(
        nc, in_maps, core_ids=list(range(NCORES)), trace=TRACE
    )
    LAST_RESULTS = res

    y = np.concatenate(
        [np.asarray(res.results[c]["y"]) for c in range(NCORES)], axis=0
    )
    return y.reshape(BATCH, SEQ, D).astype(np.float32)


# revision 19
# speedup vs baseline: 1.1698x; 1.1698x over previous
"""LoRA linear kernel for Trainium2 (8 NeuronCores, SPMD data-parallel).

Computes y = x @ (B @ A)^T for
    x: [4, 2048, 4096] f32, B: [4096, 16] f32, A: [16, 4096] f32.

Strategy: never materialize W = B @ A.  Factor as t = x @ A^T (rank 16)
then y = t @ B^T.  Tokens (4*2048 = 8192) are sharded across 8 cores
(1024 tokens each); A and B are replicated.

The kernel is HBM-bandwidth bound (~358 GB/s/core), so x is staged and
y is returned in float16 (tolerance is 2e-2; fp16 end-to-end gives
~7e-4), halving HBM traffic vs fp32.  Matmuls run fp16 x fp16 with
fp32 PSUM accumulation; y is cast to fp16 during PSUM evacuation.

DMA-descriptor discipline (the descriptor structure follows the SBUF
tile's innermost contiguous run, so):
  - x is staged chunk-major and loaded into FLAT [128, 8*512] tiles
    -> 8 KiB per-partition descriptors (near line rate).
  - at is staged flat [128, 512] (not [128, 32, 16], which would emit
    4096 32-byte descriptors).
  - y rows are [128, 4096] -> 8 KiB descriptors.

PSUM evacuation: two mm2 n-slices per 2-bank PSUM tile, evacuated by
single [128, 1024] fp32->fp16 copies alternating DVE/ACT.  y DMAs are
issued from the Sync sequencer: an ACT-issued DMA would sit in ACT's
strict-FIFO queue waiting on DVE's copies, stalling ACT's own copies.

Per-core dataflow:
  mm1: t^T[16, tok]  = sum_ko  A^T[ko]  (lhsT [128,16]) . x^T[ko] (rhs [128,512])
  mm2: y[tok128, o]  = t^T[:, chunk] (lhsT [16,128])    . B^T     (rhs [16,512])
  y DMA'd out in natural token-major layout -> host just concatenates.
"""

import sys

import numpy as np

if "/opt/trn_rl_repo" not in sys.path:
    sys.path.insert(0, "/opt/trn_rl_repo")

# Problem shape (hardcoded per contract)
BATCH = 4
SEQ = 2048
D = 4096          # in_features == out_features
R = 16            # lora rank
NCORES = 8
NTOK = BATCH * SEQ            # 8192 tokens total
TOK = NTOK // NCORES          # 1024 tokens per core
P = 128                       # partitions
KO = D // P                   # 32 feature chunks
TB = 512                      # token block for mm1 (psum bank: 512 fp32)
NB = 512                      # mm2 free dim (psum bank: 512 fp32)
KC = 8                        # ko chunks per x DMA (8 KiB/partition)
NCH = KO // KC                # x DMA chunks per token block

# Module-level knobs for test.py (harness never touches these)
TRACE = False
LAST_RESULTS = None

_nc_cache = None


def _build_program():
    from concourse import bacc, mybir, tile

    # Bacc (not raw Bass): its finalize() runs generate_event_semaphores,
    # which splits multi-sem waits to satisfy TRN2's 1-wait-per-instruction
    # hardware constraint (walrus rejects >1 otherwise).
    nc = bacc.Bacc(
        "TRN2", target_bir_lowering=False, debug=False, num_devices=NCORES
    )

    f32 = mybir.dt.float32
    f16 = mybir.dt.float16

    n_blocks = TOK // TB
    xt = nc.dram_tensor(
        "xt", [n_blocks * NCH, P, KC * TB], f16, kind="ExternalInput"
    )
    at = nc.dram_tensor("at", [P, KO * R], f16, kind="ExternalInput")
    # bt128: B^T at partition rows {32g..32g+15}, ZEROS elsewhere.  mm2
    # contracts K=128: the four rank-16 bands of t sum into y, and the
    # zero rows null out whatever sits in t's gap partitions.
    bt = nc.dram_tensor("bt", [P, D], f16, kind="ExternalInput")
    y = nc.dram_tensor("y", [TOK, D], f16, kind="ExternalOutput")

    with tile.TileContext(nc) as tc:
        with (
            tc.tile_pool(name="consts", bufs=1) as consts,
            tc.tile_pool(name="xin", bufs=2 * NCH) as xin,
            tc.tile_pool(name="tbuf", bufs=2) as tbuf,
            tc.tile_pool(name="yout", bufs=6) as yout,
            tc.tile_pool(name="pt", bufs=2, space="PSUM") as pt_pool,
            tc.tile_pool(name="py", bufs=3, space="PSUM") as py_pool,
        ):
            at_s = consts.tile([P, KO * R], f16)
            nc.sync.dma_start(at_s[:], at[:])
            bt_s = consts.tile([P, D], f16)
            nc.sync.dma_start(bt_s[:], bt[:])

            # Warm-up matmuls: (a) make PE observe the at/bt DMA sems early,
            # (b) keep PE streaming during the x-DMA prologue so the HAM
            # clock gate reaches K=8/8 before the real matmuls start,
            # (c) overwrite both pt-pool PSUM banks with finite values so
            # the gap partitions cast to finite fp16 in round_t.
            obs1 = py_pool.tile([R, R], f32, tag="psum_y")
            nc.tensor.matmul(obs1[:], at_s[:, :R], at_s[:, :R], start=True, stop=True)
            for _ in range(2):
                warm_t = pt_pool.tile([P, TB], f32, tag="psum_t")
                nc.tensor.matmul(warm_t[:], bt_s[:, :P], bt_s[:, :TB], start=True, stop=True)
            for _ in range(2):
                warm = py_pool.tile([P, 2, NB], f32, tag="psum_y")
                nc.tensor.matmul(warm[:, 0, :], bt_s[:, :P], bt_s[:, :NB], start=True, stop=True)
            tc.no_sync_barrier()

            def load_x(tb):
                xts = []
                for kc in range(NCH):
                    xt_tile = xin.tile([P, KC * TB], f16, tag="xt")
                    nc.sync.dma_start(xt_tile[:], xt[tb * NCH + kc])
                    xts.append(xt_tile)
                return xts

            def mm1_chunk(xts, psum_t4, kc):
                # 4-way column tiling: 4 concurrent rank-16 matmuls in the
                # 32-column strips of the PE array.  Band j of psum_t4
                # (rows 32j..32j+16) accumulates ko = j (mod 4) partials.
                # One PSUM bank for all bands: only the very first matmul
                # carries start=True (it clears has_written for the whole
                # bank; the other bands' first writes land on cleared bits
                # and therefore overwrite, which is exactly right).
                for g in range(KC // 4):
                    for j in range(4):
                        ko = kc * KC + g * 4 + j
                        nc.tensor.matmul(
                            psum_t4[32 * j : 32 * j + R, :],
                            at_s[:, ko * R : (ko + 1) * R],
                            xts[kc][:, (g * 4 + j) * TB : (g * 4 + j + 1) * TB],
                            start=(ko == 0),
                            stop=(ko == KO - 1),
                            tile_position=(0, 32 * j),
                            skip_group_check=True,
                        )

            def mm1(xts, psum_t4):
                for kc in range(NCH):
                    mm1_chunk(xts, psum_t4, kc)

            def round_t(psum_t4):
                # fp32 PSUM -> fp16 SBUF, partition-preserving (the gap
                # partitions carry finite garbage that bt128's zero rows
                # null out in mm2).  Halves split across DVE and ACT.
                tT4 = tbuf.tile([P, TB], f16)
                nc.vector.tensor_copy(tT4[: P // 2, :], psum_t4[: P // 2, :])
                nc.scalar.copy(tT4[P // 2 :, :], psum_t4[P // 2 :, :])
                return tT4

            def mm2_chunk(tb, c, tT):
                y_row = yout.tile([P, D], f16)
                for pair in range(D // (2 * NB)):
                    # Two n-slices into one 2-bank PSUM tile, evacuated by a
                    # single [128, 1024] fp32->fp16 copy (amortizes the
                    # fixed per-op PSUM-read cost).
                    psum_y = py_pool.tile([P, 2, NB], f32, tag="psum_y")
                    for k in range(2):
                        n = 2 * pair + k
                        nc.tensor.matmul(
                            psum_y[:, k, :],
                            tT[:, c * P : (c + 1) * P],
                            bt_s[:, n * NB : (n + 1) * NB],
                            start=True,
                            stop=True,
                        )
                    # Alternate PSUM-evacuation between DVE and ACT so
                    # neither engine gates the tensor engine's psum slots
                    n0 = 2 * pair * NB
                    if pair % 2 == 1:
                        nc.scalar.copy(y_row[:, n0 : n0 + 2 * NB], psum_y[:])
                    else:
                        nc.vector.tensor_copy(y_row[:, n0 : n0 + 2 * NB], psum_y[:])
                row0 = tb * TB + c * P
                # GpSimd (SWDGE) ring: x owns the Sync ring (a y DMA behind
                # 8 MB of queued x descriptors would serialize), and an
                # ACT-issued DMA would block ACT's own copies in its FIFO.
                nc.gpsimd.dma_start(y[row0 : row0 + P, :], y_row[:])

            # PE order must follow x-arrival order (PE is FIFO: a matmul
            # waiting on a late DMA blocks everything behind it).  mm1 of
            # block b+1 is interleaved between mm2 chunks of block b so the
            # PE instruction stream stays dense and block b+1's t is ready
            # the moment block b's y chunks finish.
            xts0 = load_x(0)
            xts1 = load_x(1)
            psum_t0 = pt_pool.tile([P, TB], f32, tag="psum_t")
            mm1(xts0, psum_t0)
            tT0 = round_t(psum_t0)
            psum_t1 = pt_pool.tile([P, TB], f32, tag="psum_t")
            for c in range(TB // P):
                mm2_chunk(0, c, tT0)
                mm1_chunk(xts1, psum_t1, c)
            tT1 = round_t(psum_t1)
            for c in range(TB // P):
                mm2_chunk(1, c, tT1)

    nc.finalize()
    return nc


def kernel(x, lora_matrix_B, lora_matrix_A):
    global _nc_cache, LAST_RESULTS
    from concourse.bass_utils import run_bass_kernel_spmd

    if _nc_cache is None:
        _nc_cache = _build_program()
    nc = _nc_cache

    x_flat = np.asarray(x, dtype=np.float32).reshape(NTOK, D).astype(np.float16)
    A = np.asarray(lora_matrix_A, dtype=np.float32).astype(np.float16)
    B = np.asarray(lora_matrix_B, dtype=np.float32).astype(np.float16)

    # at[p, ko*R + j] = A[j, ko*128 + p];  bt[32g + j, o] = B[o, j], 0 in gaps
    at_prep = np.ascontiguousarray(
        A.reshape(R, KO, P).transpose(2, 1, 0).reshape(P, KO * R)
    )
    bt_prep = np.zeros((P, D), dtype=np.float16)
    for g in range(4):
        bt_prep[32 * g : 32 * g + R, :] = B.T

    n_blocks = TOK // TB
    in_maps = []
    for c in range(NCORES):
        xc = x_flat[c * TOK : (c + 1) * TOK, :]
        # xt[(tb, kc), p, j*TB + t] = xc[tb*TB + t, (kc*KC + j)*128 + p]
        xt_prep = np.ascontiguousarray(
            xc.reshape(n_blocks, TB, NCH, KC, P).transpose(0, 2, 4, 3, 1)
        ).reshape(n_blocks * NCH, P, KC * TB)
        in_maps.append({"xt": xt_prep, "at": at_prep, "bt": bt_prep})

    res = run_bass_kernel_spmd(
        nc, in_maps, core_ids=list(range(NCORES)), trace=TRACE
    )
    LAST_RESULTS = res

    y = np.concatenate(
        [np.asarray(res.results[c]["y"]) for c in range(NCORES)], axis=0
    )
    return y.reshape(BATCH, SEQ, D).astype(np.float32)


# revision 26
# speedup vs baseline: 1.2042x; 1.0294x over previous
"""LoRA linear kernel for Trainium2 (8 NeuronCores, SPMD data-parallel).

Computes y = x @ (B @ A)^T for
    x: [4, 2048, 4096] f32, B: [4096, 16] f32, A: [16, 4096] f32.

Strategy: never materialize W = B @ A.  Factor as t = x @ A^T (rank 16)
then y = t @ B^T.  Tokens (4*2048 = 8192) are sharded across 8 cores
(1024 tokens each); A and B are replicated.

The kernel is HBM-bandwidth bound (~358 GB/s/core), so x is staged and
y is returned in float16 (tolerance is 2e-2; fp16 end-to-end gives
~8e-4), halving HBM traffic vs fp32.

Pipeline: 8 independent chunks of 128 tokens.  Per chunk (1 MiB in,
1 MiB out) the dataflow is x-DMA -> mm1 -> cast -> mm2 -> cast -> y-DMA
with ~4 us latency, so y writes start streaming while x is still
loading and the HBM bus never idles.

mm1 is 4-way column-tiled: four concurrent rank-16 matmuls in the
32-column strips of the PE array (tile_position), accumulating
ko = j (mod 4) partials into the four 16-row bands of one PSUM bank.
Only the global first matmul carries start=True (it clears has_written
for the whole bank; the other bands' first writes land on cleared bits
and overwrite).  mm2 contracts K=128 against bt128 = B^T replicated at
partition rows {32g..32g+15} with ZEROS elsewhere: the four bands sum
into y and the zero rows null the garbage in t's gap partitions
(pre-filled with finite values by the warm-up matmuls).  PSUM->SBUF
casts are partition-preserving and split across DVE and ACT.

DMA-descriptor discipline (descriptor structure follows the SBUF
tile's innermost contiguous run): x chunks and y rows are flat
[128, 8 KiB] tiles -> 8 KiB per-partition descriptors (line rate).
x owns the Sync HWDGE ring; bt + y own the GpSimd SWDGE ring (an
ACT-issued y DMA would stall ACT's own copies in its FIFO queue, and
on the Sync ring it would queue behind 8 MiB of x descriptors).
"""

import sys

import numpy as np

if "/opt/trn_rl_repo" not in sys.path:
    sys.path.insert(0, "/opt/trn_rl_repo")

# Problem shape (hardcoded per contract)
BATCH = 4
SEQ = 2048
D = 4096          # in_features == out_features
R = 16            # lora rank
NCORES = 8
NTOK = BATCH * SEQ            # 8192 tokens total
TOK = NTOK // NCORES          # 1024 tokens per core
P = 128                       # partitions
KO = D // P                   # 32 feature chunks
TC = 128                      # tokens per pipeline chunk
NCH = TOK // TC               # 8 chunks per core
NB = 512                      # mm2 free dim (psum bank: 512 fp32)

# Module-level knobs for test.py (harness never touches these)
TRACE = False
LAST_RESULTS = None

_nc_cache = None


def _build_program():
    from concourse import bacc, mybir, tile

    # Bacc (not raw Bass): its finalize() runs generate_event_semaphores,
    # which splits multi-sem waits to satisfy TRN2's 1-wait-per-instruction
    # hardware constraint (walrus rejects >1 otherwise).
    nc = bacc.Bacc(
        "TRN2", target_bir_lowering=False, debug=False, num_devices=NCORES
    )

    f32 = mybir.dt.float32
    f16 = mybir.dt.float16

    xt = nc.dram_tensor("xt", [NCH, P, KO * TC], f16, kind="ExternalInput")
    # at carries a trailing [P, P] ZERO block: the stationary operand of
    # each chunk's bank-clearing dummy matmul.
    at = nc.dram_tensor("at", [P, KO * R + P], f16, kind="ExternalInput")
    bt = nc.dram_tensor("bt", [P, D], f16, kind="ExternalInput")
    y = nc.dram_tensor("y", [TOK, D], f16, kind="ExternalOutput")

    with tile.TileContext(nc) as tc:
        with (
            tc.tile_pool(name="consts", bufs=1) as consts,
            tc.tile_pool(name="xin", bufs=NCH) as xin,
            tc.tile_pool(name="tbuf", bufs=2) as tbuf,
            tc.tile_pool(name="yout", bufs=6) as yout,
            tc.tile_pool(name="pt", bufs=2, space="PSUM") as pt_pool,
            tc.tile_pool(name="py", bufs=3, space="PSUM") as py_pool,
        ):
            at_s = consts.tile([P, KO * R + P], f16)
            nc.sync.dma_start(at_s[:], at[:])
            bt_s = consts.tile([P, D], f16)
            nc.sync.dma_start(bt_s[:], bt[:])

            # Warm-up matmuls (at-based, so they only gate on the tiny at
            # DMA): (a) keep PE streaming during the x prologue so the HAM
            # clock gate reaches K=8/8 early, (b) overwrite both pt-pool
            # PSUM banks with finite values so the gap partitions cast to
            # finite fp16 in round_t.
            for _ in range(2):
                warm_t = pt_pool.tile([P, TC], f32, tag="psum_t")
                nc.tensor.matmul(warm_t[:], at_s[:, :P], at_s[:, :TC], start=True, stop=True)
            for _ in range(2):
                warm = py_pool.tile([P, 2, NB], f32, tag="psum_y")
                nc.tensor.matmul(warm[:, 0, :], at_s[:, :P], at_s[:, :NB], start=True, stop=True)
            tc.no_sync_barrier()

            def mm1_chunk(xt_tile, psum_t):
                # Bank-clearing dummy: zero lhsT, full [128, TC] output.
                # Clears has_written for the whole bank, zero-fills every
                # row (gap rows included), and its write-after-write overlap
                # with every band forces the scheduler to order it first.
                # The 32 real matmuls then all accumulate (start=False),
                # which is order-independent.
                nc.tensor.matmul(
                    psum_t[:],
                    at_s[:, KO * R : KO * R + P],
                    xt_tile[:, :TC],
                    start=True,
                    stop=False,
                    skip_group_check=True,
                )
                # 4-way column tiling: 4 concurrent rank-16 matmuls; band j
                # (psum rows 32j..32j+16) accumulates ko = j (mod 4).
                for g in range(KO // 4):
                    for j in range(4):
                        ko = g * 4 + j
                        nc.tensor.matmul(
                            psum_t[32 * j : 32 * j + R, :],
                            at_s[:, ko * R : (ko + 1) * R],
                            xt_tile[:, ko * TC : (ko + 1) * TC],
                            start=False,
                            stop=(ko == KO - 1),
                            tile_position=(0, 32 * j),
                            skip_group_check=True,
                        )

            def round_t(psum_t):
                # fp32 PSUM -> fp16 SBUF, partition-preserving (bt128's
                # zero rows null the gap partitions in mm2).
                tT = tbuf.tile([P, TC], f16)
                nc.vector.tensor_copy(tT[: P // 2, :], psum_t[: P // 2, :])
                nc.scalar.copy(tT[P // 2 :, :], psum_t[P // 2 :, :])
                return tT

            def mm2_chunk(c, tT):
                y_row = yout.tile([P, D], f16)
                for pair in range(D // (2 * NB)):
                    # Two n-slices into one 2-bank PSUM tile, evacuated by
                    # a single [128, 1024] fp32->fp16 copy.
                    psum_y = py_pool.tile([P, 2, NB], f32, tag="psum_y")
                    for k in range(2):
                        n = 2 * pair + k
                        nc.tensor.matmul(
                            psum_y[:, k, :],
                            tT[:],
                            bt_s[:, n * NB : (n + 1) * NB],
                            start=True,
                            stop=True,
                        )
                    n0 = 2 * pair * NB
                    if pair % 2 == 1:
                        nc.scalar.copy(y_row[:, n0 : n0 + 2 * NB], psum_y[:])
                    else:
                        nc.vector.tensor_copy(y_row[:, n0 : n0 + 2 * NB], psum_y[:])
                nc.gpsimd.dma_start(y[c * TC : (c + 1) * TC, :], y_row[:])

            # Uniform 128-token pipeline; emission order == x arrival order
            # (PE is FIFO: a matmul waiting on a late DMA blocks everything
            # behind it).
            for c in range(NCH):
                xt_tile = xin.tile([P, KO * TC], f16, tag="xt")
                nc.sync.dma_start(xt_tile[:], xt[c])
                psum_t = pt_pool.tile([P, TC], f32, tag="psum_t")
                mm1_chunk(xt_tile, psum_t)
                tT = round_t(psum_t)
                mm2_chunk(c, tT)

    nc.finalize()
    return nc


def kernel(x, lora_matrix_B, lora_matrix_A):
    global _nc_cache, LAST_RESULTS
    from concourse.bass_utils import run_bass_kernel_spmd

    if _nc_cache is None:
        _nc_cache = _build_program()
    nc = _nc_cache

    x_flat = np.asarray(x, dtype=np.float32).reshape(NTOK, D).astype(np.float16)
    A = np.asarray(lora_matrix_A, dtype=np.float32).astype(np.float16)
    B = np.asarray(lora_matrix_B, dtype=np.float32).astype(np.float16)

    # at[p, ko*R + j] = A[j, ko*128 + p], then a [P, P] zero block;
    # bt[32g + j, o] = B[o, j], 0 in gaps
    at_prep = np.zeros((P, KO * R + P), dtype=np.float16)
    at_prep[:, : KO * R] = A.reshape(R, KO, P).transpose(2, 1, 0).reshape(P, KO * R)
    bt_prep = np.zeros((P, D), dtype=np.float16)
    for g in range(4):
        bt_prep[32 * g : 32 * g + R, :] = B.T

    in_maps = []
    for c in range(NCORES):
        xc = x_flat[c * TOK : (c + 1) * TOK, :]
        # xt[ch, p, ko*TC + t] = xc[ch*TC + t, ko*128 + p]
        xt_prep = np.ascontiguousarray(
            xc.reshape(NCH, TC, KO, P).transpose(0, 3, 2, 1)
        ).reshape(NCH, P, KO * TC)
        in_maps.append({"xt": xt_prep, "at": at_prep, "bt": bt_prep})

    res = run_bass_kernel_spmd(
        nc, in_maps, core_ids=list(range(NCORES)), trace=TRACE
    )
    LAST_RESULTS = res

    y = np.concatenate(
        [np.asarray(res.results[c]["y"]) for c in range(NCORES)], axis=0
    )
    return y.reshape(BATCH, SEQ, D).astype(np.float32)
